# revision 10
# baseline (speedup 1.0000x reference)
"""Partial trace kernel for Trainium2 (8 NeuronCores, SPMD).

Problem: rho is (1,1,8192,8192) fp32 = a 13-qubit density matrix (D=2, N=13).
Output: partial trace keeping qubits Q=(3,7,11) -> (8,8) fp32.

Math: out[a,b] = sum_t rho[row(a,t), col(b,t)] where a,b are the 3 kept-qubit
bits (weights 1024/64/4 in the 13-bit index, big-endian qubit order) and t
ranges over the 1024 configurations of the 10 traced qubits (row bits == col
bits on the diagonal). Only 65536 of the 67M elements of rho contribute.

Sharding (per the row-block hint): core a takes the 1024 rows whose kept-qubit
bits equal a, reordered so that slice-row = compact traced index t.  Each core
then gathers its 8192 needed elements (8 output columns x 1024 diagonal terms)
with a single strided-AP DMA, reduces on-chip, and writes its 8-value output
row.  No collective needed: output rows are disjoint across cores.

Inside the per-core slice x[1024, 8192] the needed element (b, t) sits at flat
index  boff(b) + sum_k t_k * SK[k]  with SK[k] = 2^k * 8192 + WT[k]
(row stride 2^k * 8192 plus the traced qubit's column weight WT[k]).
"""

import numpy as np

_WK = (1024, 64, 4)  # row/col weights of kept qubits 3, 7, 11 (2^(13-q))
_WT = (1, 2, 8, 16, 32, 128, 256, 512, 2048, 4096)  # traced weights, ascending
_NCORES = 8
_T = 1024  # traced configurations
_state = {}

# test-harness hooks (the grading harness never touches these)
TRACE = False
LAST_RESULTS = None


def _rowmask():
    t = np.arange(_T)
    rm = np.zeros(_T, np.int64)
    for k in range(10):
        rm += ((t >> k) & 1) * _WT[k]
    return rm


def _build_nc():
    import concourse.bass as bass
    import concourse.mybir as mybir

    nc = bass.Bass()
    f32 = mybir.dt.float32
    x = nc.dram_tensor("x", [_T, 8192], f32, kind="ExternalInput")
    out = nc.dram_tensor("out", [8], f32, kind="ExternalOutput")

    # per-slice element strides of the 10 traced bits
    SK = [(1 << k) * 8192 + _WT[k] for k in range(10)]

    # DMA APs allow only 3 balanced dims and the inner dim must be contiguous.
    # So each descriptor is a 5-element contiguous run
    # [b0=0, junk, junk, junk, b0=1] (20B instead of 2 x 4B scattered), and
    # each DMA moves chains (t7t6t5) x (t4t3t2) x run5 = 320 elems.
    # Chunks: (b2b1) x (t9t8) x (t1t0) = 64 DMAs, 64 descriptors each,
    # split across the two HWDGE rings (sync/SP and scalar/Activation).
    # Layout: p = (b2b1)*32 + (t9t8)*8 + (t7t6t5), f = (t1t0)*40 + (t4t3t2)*5 + run
    chunks = []
    for bb in range(4):
        for t98 in range(4):
            for t10 in range(4):
                src0 = (
                    (bb >> 1) * _WK[0]
                    + (bb & 1) * _WK[1]
                    + (t98 >> 1) * SK[9]
                    + (t98 & 1) * SK[8]
                    + (t10 >> 1) * SK[1]
                    + (t10 & 1) * SK[0]
                )
                chunks.append((src0, bb * 32 + t98 * 8, t10 * 40))

    with (
        nc.sbuf_tensor("g", [128, 160], f32) as g,
        nc.sbuf_tensor("rr", [128, 2], f32) as rr,
        nc.sbuf_tensor("sel", [128, 4], f32) as sel,
        nc.psum_tensor("acc", [4, 2], f32) as acc,
        nc.sbuf_tensor("o", [4, 2], f32) as o,
        nc.semaphore("dma_sem") as dma_sem,
        nc.semaphore("v_sem") as v_sem,
        nc.semaphore("mm_sem") as mm_sem,
        nc.semaphore("o_sem") as o_sem,
        nc.Block() as block,
    ):

        def emit_gather(eng, part):
            for src0, p0, f0 in chunks[part::2]:
                src = bass.AP(x, src0, [[SK[5], 8], [SK[2], 8], [1, 5]])
                dst = g[p0 : p0 + 8, f0 : f0 + 40].rearrange("p (i r) -> p i r", r=5)
                eng.dma_start(out=dst, in_=src).then_inc(dma_sem, 16)

        @block.sync
        def _(sync):
            emit_gather(sync, 0)
            sync.wait_ge(o_sem, 1)
            sync.dma_start(out=out[:], in_=o[:, :]).then_inc(dma_sem, 16)
            sync.wait_ge(dma_sem, 65 * 16)

        @block.scalar
        def _(scalar):
            emit_gather(scalar, 1)

        @block.vector
        def _(vector):
            # selector for the partition-group reduce (built while DMAs fly)
            vector.memset(sel[:, :], 0.0)
            for j in range(4):
                vector.memset(sel[32 * j : 32 * (j + 1), j : j + 1], 1.0)
            vector.wait_ge(dma_sem, 64 * 16)
            # rr[p, c] = sum over (t1t0, t4t3t2) of g at run position 4*c
            gv = g[:, :].rearrange("p (t i r) -> p t i r", i=8, r=5)
            for c in range(2):
                ins = vector.tensor_reduce(
                    rr[:, c : c + 1],
                    gv[:, :, :, 4 * c],
                    axis=mybir.AxisListType.XY,
                    op=mybir.AluOpType.add,
                )
            ins.then_inc(v_sem, 1)
            vector.wait_ge(mm_sem, 1)
            vector.tensor_copy(o[:, :], acc[:, :]).then_inc(o_sem, 1)

        @block.tensor
        def _(tensor):
            # acc[j, c] = sum_{p>>5 == j} rr[p, c]
            tensor.wait_ge(v_sem, 1)
            tensor.matmul(acc[:, :], sel[:, :], rr[:, :], start=True, stop=True).then_inc(
                mm_sem, 1
            )

    nc.finalize()
    return nc


def _shard(rho):
    """core a -> rows with kept bits == a, ordered by compact traced index."""
    rm = _rowmask()
    maps = []
    for a in range(_NCORES):
        base = ((a >> 2) & 1) * _WK[0] + ((a >> 1) & 1) * _WK[1] + (a & 1) * _WK[2]
        maps.append({"x": np.ascontiguousarray(rho[base + rm, :])})
    return maps


def kernel(rho):
    global LAST_RESULTS
    from concourse.bass_utils import run_bass_kernel_spmd

    rho = np.asarray(rho, dtype=np.float32).reshape(8192, 8192)
    if "nc" not in _state:
        _state["nc"] = _build_nc()
    res = run_bass_kernel_spmd(
        _state["nc"], _shard(rho), list(range(_NCORES)), trace=TRACE
    )
    LAST_RESULTS = res
    return np.stack([res.results[a]["out"] for a in range(_NCORES)]).astype(np.float32)


# revision 11
# speedup vs baseline: 1.5575x; 1.5575x over previous
"""Partial trace kernel for Trainium2 (8 NeuronCores, SPMD).

Problem: rho is (1,1,8192,8192) fp32 = a 13-qubit density matrix (D=2, N=13).
Output: partial trace keeping qubits Q=(3,7,11) -> (8,8) fp32.

Math: out[a,b] = sum_t rho[row(a,t), col(b,t)] where a,b are the 3 kept-qubit
bits (weights 1024/64/4 in the 13-bit index, big-endian qubit order) and t
ranges over the 1024 configurations of the 10 traced qubits (row bits == col
bits on the diagonal). Only 65536 of the 67M elements of rho contribute.

Sharding (per the row-block hint): core a takes the 1024 rows whose kept-qubit
bits equal a, reordered so that slice-row = compact traced index t.  Each core
then gathers its 8192 needed elements (8 output columns x 1024 diagonal terms)
with a single strided-AP DMA, reduces on-chip, and writes its 8-value output
row.  No collective needed: output rows are disjoint across cores.

Inside the per-core slice x[1024, 8192] the needed element (b, t) sits at flat
index  boff(b) + sum_k t_k * SK[k]  with SK[k] = 2^k * 8192 + WT[k]
(row stride 2^k * 8192 plus the traced qubit's column weight WT[k]).
"""

import numpy as np

_WK = (1024, 64, 4)  # row/col weights of kept qubits 3, 7, 11 (2^(13-q))
_WT = (1, 2, 8, 16, 32, 128, 256, 512, 2048, 4096)  # traced weights, ascending
_NCORES = 8
_T = 1024  # traced configurations
_state = {}

# test-harness hooks (the grading harness never touches these)
TRACE = False
LAST_RESULTS = None


def _rowmask():
    t = np.arange(_T)
    rm = np.zeros(_T, np.int64)
    for k in range(10):
        rm += ((t >> k) & 1) * _WT[k]
    return rm


def _build_nc():
    import concourse.bass as bass
    import concourse.mybir as mybir

    nc = bass.Bass()
    f32 = mybir.dt.float32
    x = nc.dram_tensor("x", [_T, 8192], f32, kind="ExternalInput")
    out = nc.dram_tensor("out", [8], f32, kind="ExternalOutput")

    # per-slice element strides of the 10 traced bits
    SK = [(1 << k) * 8192 + _WT[k] for k in range(10)]

    # DMA APs allow only 3 balanced dims and the inner dim must be contiguous.
    # Each descriptor is a 69-element contiguous run covering the 4 needed
    # values at offsets {0,4,64,68} (= b1,b0 combos) for one b2 — 276B per
    # descriptor, 2048 descriptors total (the issuing engines are the
    # bottleneck at ~10.5ns/descriptor, so fewer descriptors wins even with
    # 17x junk).  Each DMA moves chains (t7t6t5) x (t4t3t2) x run69;
    # chunks (b2, t9, t8, t1, t0) = 32 DMAs, 64 descriptors each, split
    # across the three descriptor generators (sync/SP, scalar/Act, gpsimd).
    # Layout: p = b2*64 + t9*32 + t8*16 + t1*8 + (t7t6t5),
    #         f = t0*552 + (t4t3t2)*69 + run
    chunks = []
    for b2 in range(2):
        for t9 in range(2):
            for t8 in range(2):
                for t1 in range(2):
                    for t0 in range(2):
                        src0 = (
                            b2 * _WK[0]
                            + t9 * SK[9]
                            + t8 * SK[8]
                            + t1 * SK[1]
                            + t0 * SK[0]
                        )
                        p0 = b2 * 64 + t9 * 32 + t8 * 16 + t1 * 8
                        chunks.append((src0, p0, t0 * 552))
    RPOS = (0, 4, 64, 68)  # run offsets of (b1,b0) = (0,0),(0,1),(1,0),(1,1)
    N_DMA = len(chunks)

    with (
        nc.sbuf_tensor("g", [128, 1104], f32) as g,
        nc.sbuf_tensor("rr", [128, 4], f32) as rr,
        nc.sbuf_tensor("sel", [128, 2], f32) as sel,
        nc.psum_tensor("acc", [2, 4], f32) as acc,
        nc.sbuf_tensor("o", [2, 4], f32) as o,
        nc.semaphore("dma_sem") as dma_sem,
        nc.semaphore("v_sem") as v_sem,
        nc.semaphore("mm_sem") as mm_sem,
        nc.semaphore("o_sem") as o_sem,
        nc.Block() as block,
    ):
        # chunk index ranges per issuing engine (tuned from trace)
        SYNC_N, SCAL_N = 13, 13  # gpsimd gets the rest

        def emit_gather(eng, lo, hi):
            for src0, p0, f0 in chunks[lo:hi]:
                src = bass.AP(x, src0, [[SK[5], 8], [SK[2], 8], [1, 69]])
                dst = g[p0 : p0 + 8, f0 : f0 + 552].rearrange("p (i r) -> p i r", r=69)
                eng.dma_start(out=dst, in_=src).then_inc(dma_sem, 16)

        @block.sync
        def _(sync):
            emit_gather(sync, 0, SYNC_N)
            sync.wait_ge(o_sem, 1)
            sync.dma_start(out=out[:], in_=o[:, :]).then_inc(dma_sem, 16)
            sync.wait_ge(dma_sem, (N_DMA + 1) * 16)

        @block.scalar
        def _(scalar):
            emit_gather(scalar, SYNC_N, SYNC_N + SCAL_N)

        @block.gpsimd
        def _(gpsimd):
            emit_gather(gpsimd, SYNC_N + SCAL_N, N_DMA)

        @block.vector
        def _(vector):
            # selector for the partition-group reduce (built while DMAs fly)
            vector.memset(sel[:, :], 0.0)
            vector.memset(sel[0:64, 0:1], 1.0)
            vector.memset(sel[64:128, 1:2], 1.0)
            vector.wait_ge(dma_sem, N_DMA * 16)
            # rr[p, j] = sum over (t0, t4t3t2) of g at run position RPOS[j]
            gv = g[:, :].rearrange("p (t i r) -> p t i r", i=8, r=69)
            for j in range(4):
                ins = vector.tensor_reduce(
                    rr[:, j : j + 1],
                    gv[:, :, :, RPOS[j]],
                    axis=mybir.AxisListType.XY,
                    op=mybir.AluOpType.add,
                )
            ins.then_inc(v_sem, 1)
            vector.wait_ge(mm_sem, 1)
            vector.tensor_copy(o[:, :], acc[:, :]).then_inc(o_sem, 1)

        @block.tensor
        def _(tensor):
            # acc[j, :] = sum_{p>>6 == j} rr[p, :]
            tensor.wait_ge(v_sem, 1)
            tensor.matmul(acc[:, :], sel[:, :], rr[:, :], start=True, stop=True).then_inc(
                mm_sem, 1
            )

    nc.finalize()
    return nc


def _shard(rho):
    """core a -> rows with kept bits == a, ordered by compact traced index."""
    rm = _rowmask()
    maps = []
    for a in range(_NCORES):
        base = ((a >> 2) & 1) * _WK[0] + ((a >> 1) & 1) * _WK[1] + (a & 1) * _WK[2]
        maps.append({"x": np.ascontiguousarray(rho[base + rm, :])})
    return maps


def kernel(rho):
    global LAST_RESULTS
    from concourse.bass_utils import run_bass_kernel_spmd

    rho = np.asarray(rho, dtype=np.float32).reshape(8192, 8192)
    if "nc" not in _state:
        _state["nc"] = _build_nc()
    res = run_bass_kernel_spmd(
        _state["nc"], _shard(rho), list(range(_NCORES)), trace=TRACE
    )
    LAST_RESULTS = res
    return np.stack([res.results[a]["out"] for a in range(_NCORES)]).astype(np.float32)


# revision 12
# speedup vs baseline: 1.6493x; 1.0589x over previous
"""Partial trace kernel for Trainium2 (8 NeuronCores, SPMD).

Problem: rho is (1,1,8192,8192) fp32 = a 13-qubit density matrix (D=2, N=13).
Output: partial trace keeping qubits Q=(3,7,11) -> (8,8) fp32.

Math: out[a,b] = sum_t rho[row(a,t), col(b,t)] where a,b are the 3 kept-qubit
bits (weights 1024/64/4 in the 13-bit index, big-endian qubit order) and t
ranges over the 1024 configurations of the 10 traced qubits (row bits == col
bits on the diagonal). Only 65536 of the 67M elements of rho contribute.

Sharding (per the row-block hint): core a takes the 1024 rows whose kept-qubit
bits equal a, reordered so that slice-row = compact traced index t.  Each core
then gathers its 8192 needed elements (8 output columns x 1024 diagonal terms)
with a single strided-AP DMA, reduces on-chip, and writes its 8-value output
row.  No collective needed: output rows are disjoint across cores.

Inside the per-core slice x[1024, 8192] the needed element (b, t) sits at flat
index  boff(b) + sum_k t_k * SK[k]  with SK[k] = 2^k * 8192 + WT[k]
(row stride 2^k * 8192 plus the traced qubit's column weight WT[k]).
"""

import numpy as np

_WK = (1024, 64, 4)  # row/col weights of kept qubits 3, 7, 11 (2^(13-q))
_WT = (1, 2, 8, 16, 32, 128, 256, 512, 2048, 4096)  # traced weights, ascending
_NCORES = 8
_T = 1024  # traced configurations
_state = {}

# test-harness hooks (the grading harness never touches these)
TRACE = False
LAST_RESULTS = None


def _rowmask():
    t = np.arange(_T)
    rm = np.zeros(_T, np.int64)
    for k in range(10):
        rm += ((t >> k) & 1) * _WT[k]
    return rm


def _build_nc():
    import concourse.bass as bass
    import concourse.mybir as mybir

    nc = bass.Bass()
    f32 = mybir.dt.float32
    x = nc.dram_tensor("x", [_T, 8192], f32, kind="ExternalInput")
    out = nc.dram_tensor("out", [8], f32, kind="ExternalOutput")

    # per-slice element strides of the 10 traced bits
    SK = [(1 << k) * 8192 + _WT[k] for k in range(10)]

    # DMA APs allow only 3 balanced dims and the inner dim must be contiguous.
    # Each descriptor is a 69-element contiguous run covering the 4 needed
    # values at offsets {0,4,64,68} (= b1,b0 combos) for one b2 — 276B per
    # descriptor, 2048 descriptors total (the issuing engines are the
    # bottleneck at ~10.5ns/descriptor, so fewer descriptors wins even with
    # 17x junk).  Each DMA moves chains (t7t6t5) x (t4t3t2) x run69;
    # chunks (b2, t9, t8, t1, t0) = 32 DMAs, 64 descriptors each, split
    # across the three descriptor generators (sync/SP, scalar/Act, gpsimd).
    # Layout: p = b2*64 + t9*32 + t8*16 + t1*8 + (t7t6t5),
    #         f = t0*552 + (t4t3t2)*69 + run
    chunks = []
    for b2 in range(2):
        for t9 in range(2):
            for t8 in range(2):
                for t1 in range(2):
                    for t0 in range(2):
                        src0 = (
                            b2 * _WK[0]
                            + t9 * SK[9]
                            + t8 * SK[8]
                            + t1 * SK[1]
                            + t0 * SK[0]
                        )
                        p0 = b2 * 64 + t9 * 32 + t8 * 16 + t1 * 8
                        chunks.append((src0, p0, t0 * 552))
    RPOS = (0, 4, 64, 68)  # run offsets of (b1,b0) = (0,0),(0,1),(1,0),(1,1)
    N_DMA = len(chunks)

    with (
        nc.sbuf_tensor("g", [128, 1104], f32) as g,
        nc.sbuf_tensor("rr", [128, 4], f32) as rr,
        nc.sbuf_tensor("sel", [128, 2], f32) as sel,
        nc.psum_tensor("acc", [2, 4], f32) as acc,
        nc.sbuf_tensor("o", [2, 4], f32) as o,
        nc.semaphore("dma_sem") as dma_sem,
        nc.semaphore("v_sem") as v_sem,
        nc.semaphore("mm_sem") as mm_sem,
        nc.semaphore("o_sem") as o_sem,
        nc.Block() as block,
    ):
        # chunk index ranges per issuing engine (tuned from trace: each DMA
        # occupies its issuing engine ~700ns regardless of engine, so split
        # the 32 chunks evenly across the three descriptor generators)
        SYNC_N, SCAL_N = 11, 11  # gpsimd gets the rest (10)

        def emit_gather(eng, lo, hi):
            for src0, p0, f0 in chunks[lo:hi]:
                src = bass.AP(x, src0, [[SK[5], 8], [SK[2], 8], [1, 69]])
                dst = g[p0 : p0 + 8, f0 : f0 + 552].rearrange("p (i r) -> p i r", r=69)
                eng.dma_start(out=dst, in_=src).then_inc(dma_sem, 16)

        @block.sync
        def _(sync):
            emit_gather(sync, 0, SYNC_N)
            sync.wait_ge(o_sem, 1)
            sync.dma_start(out=out[:], in_=o[:, :]).then_inc(dma_sem, 16)
            sync.wait_ge(dma_sem, (N_DMA + 1) * 16)

        @block.scalar
        def _(scalar):
            emit_gather(scalar, SYNC_N, SYNC_N + SCAL_N)

        @block.gpsimd
        def _(gpsimd):
            emit_gather(gpsimd, SYNC_N + SCAL_N, N_DMA)

        @block.vector
        def _(vector):
            # selector for the partition-group reduce (built while DMAs fly)
            vector.memset(sel[:, :], 0.0)
            vector.memset(sel[0:64, 0:1], 1.0)
            vector.memset(sel[64:128, 1:2], 1.0)
            vector.wait_ge(dma_sem, N_DMA * 16)
            # rr[p, j] = sum over (t0, t4t3t2) of g at run position RPOS[j]
            gv = g[:, :].rearrange("p (t i r) -> p t i r", i=8, r=69)
            for j in range(4):
                ins = vector.tensor_reduce(
                    rr[:, j : j + 1],
                    gv[:, :, :, RPOS[j]],
                    axis=mybir.AxisListType.XY,
                    op=mybir.AluOpType.add,
                )
            ins.then_inc(v_sem, 1)
            vector.wait_ge(mm_sem, 1)
            vector.tensor_copy(o[:, :], acc[:, :]).then_inc(o_sem, 1)

        @block.tensor
        def _(tensor):
            # acc[j, :] = sum_{p>>6 == j} rr[p, :]
            tensor.wait_ge(v_sem, 1)
            tensor.matmul(acc[:, :], sel[:, :], rr[:, :], start=True, stop=True).then_inc(
                mm_sem, 1
            )

    nc.finalize()
    return nc


def _shard(rho):
    """core a -> rows with kept bits == a, ordered by compact traced index."""
    rm = _rowmask()
    maps = []
    for a in range(_NCORES):
        base = ((a >> 2) & 1) * _WK[0] + ((a >> 1) & 1) * _WK[1] + (a & 1) * _WK[2]
        maps.append({"x": np.ascontiguousarray(rho[base + rm, :])})
    return maps


def kernel(rho):
    global LAST_RESULTS
    from concourse.bass_utils import run_bass_kernel_spmd

    rho = np.asarray(rho, dtype=np.float32).reshape(8192, 8192)
    if "nc" not in _state:
        _state["nc"] = _build_nc()
    res = run_bass_kernel_spmd(
        _state["nc"], _shard(rho), list(range(_NCORES)), trace=TRACE
    )
    LAST_RESULTS = res
    return np.stack([res.results[a]["out"] for a in range(_NCORES)]).astype(np.float32)


# revision 13
# speedup vs baseline: 1.6744x; 1.0152x over previous
"""Partial trace kernel for Trainium2 (8 NeuronCores, SPMD).

Problem: rho is (1,1,8192,8192) fp32 = a 13-qubit density matrix (D=2, N=13).
Output: partial trace keeping qubits Q=(3,7,11) -> (8,8) fp32.

Math: out[a,b] = sum_t rho[row(a,t), col(b,t)] where a,b are the 3 kept-qubit
bits (weights 1024/64/4 in the 13-bit index, big-endian qubit order) and t
ranges over the 1024 configurations of the 10 traced qubits (row bits == col
bits on the diagonal). Only 65536 of the 67M elements of rho contribute.

Sharding (per the row-block hint): core a takes the 1024 rows whose kept-qubit
bits equal a, reordered so that slice-row = compact traced index t.  Each core
then gathers its 8192 needed elements (8 output columns x 1024 diagonal terms)
with a single strided-AP DMA, reduces on-chip, and writes its 8-value output
row.  No collective needed: output rows are disjoint across cores.

Inside the per-core slice x[1024, 8192] the needed element (b, t) sits at flat
index  boff(b) + sum_k t_k * SK[k]  with SK[k] = 2^k * 8192 + WT[k]
(row stride 2^k * 8192 plus the traced qubit's column weight WT[k]).
"""

import numpy as np

_WK = (1024, 64, 4)  # row/col weights of kept qubits 3, 7, 11 (2^(13-q))
_WT = (1, 2, 8, 16, 32, 128, 256, 512, 2048, 4096)  # traced weights, ascending
_NCORES = 8
_T = 1024  # traced configurations
_state = {}

# test-harness hooks (the grading harness never touches these)
TRACE = False
LAST_RESULTS = None


def _rowmask():
    t = np.arange(_T)
    rm = np.zeros(_T, np.int64)
    for k in range(10):
        rm += ((t >> k) & 1) * _WT[k]
    return rm


def _build_nc():
    import concourse.bass as bass
    import concourse.mybir as mybir

    nc = bass.Bass()
    f32 = mybir.dt.float32
    x = nc.dram_tensor("x", [_T, 8192], f32, kind="ExternalInput")
    out = nc.dram_tensor("out", [8], f32, kind="ExternalOutput")

    # per-slice element strides of the 10 traced bits
    SK = [(1 << k) * 8192 + _WT[k] for k in range(10)]

    # DMA APs allow only 3 balanced dims and the inner dim must be contiguous.
    # Each descriptor is a 69-element contiguous run covering the 4 needed
    # values at offsets {0,4,64,68} (= b1,b0 combos) for one b2 — 276B per
    # descriptor, 2048 descriptors total (the issuing engines are the
    # bottleneck at ~10.5ns/descriptor, so fewer descriptors wins even with
    # 17x junk).  Each DMA moves chains (t7t6t5) x (t4t3t2) x run69;
    # chunks (b2, t9, t8, t1, t0) = 32 DMAs, 64 descriptors each, split
    # across the three descriptor generators (sync/SP, scalar/Act, gpsimd).
    # Layout: p = b2*64 + t9*32 + t8*16 + t1*8 + (t7t6t5),
    #         f = t0*552 + (t4t3t2)*69 + run
    chunks = []
    for b2 in range(2):
        for t9 in range(2):
            for t8 in range(2):
                for t1 in range(2):
                    for t0 in range(2):
                        src0 = (
                            b2 * _WK[0]
                            + t9 * SK[9]
                            + t8 * SK[8]
                            + t1 * SK[1]
                            + t0 * SK[0]
                        )
                        p0 = b2 * 64 + t9 * 32 + t8 * 16 + t1 * 8
                        chunks.append((src0, p0, t0 * 552))
    RPOS = (0, 4, 64, 68)  # run offsets of (b1,b0) = (0,0),(0,1),(1,0),(1,1)
    N_DMA = len(chunks)

    with (
        nc.sbuf_tensor("g", [128, 1104], f32) as g,
        nc.sbuf_tensor("rr", [128, 4], f32) as rr,
        nc.sbuf_tensor("sel", [128, 2], f32) as sel,
        nc.psum_tensor("acc", [2, 4], f32) as acc,
        nc.sbuf_tensor("o", [2, 4], f32) as o,
        nc.semaphore("dma_a") as dma_a,
        nc.semaphore("dma_b") as dma_b,
        nc.semaphore("v_sem") as v_sem,
        nc.semaphore("mm_sem") as mm_sem,
        nc.semaphore("o_sem") as o_sem,
        nc.Block() as block,
    ):
        # Round-robin chunks across the three descriptor generators (each DMA
        # occupies its issuing engine ~700ns regardless of engine).  chunks[]
        # is b2-major, so every engine issues its b2=0 (partitions 0-63)
        # chunks first; those complete early and signal dma_a so the first
        # half of the reduction hides under the b2=1 DMAs (dma_b).
        def emit_gather(eng, which):
            for ci in range(which, N_DMA, 3):
                src0, p0, f0 = chunks[ci]
                src = bass.AP(x, src0, [[SK[5], 8], [SK[2], 8], [1, 69]])
                dst = g[p0 : p0 + 8, f0 : f0 + 552].rearrange("p (i r) -> p i r", r=69)
                sem = dma_a if ci < 16 else dma_b
                eng.dma_start(out=dst, in_=src).then_inc(sem, 16)

        @block.sync
        def _(sync):
            emit_gather(sync, 0)
            sync.wait_ge(o_sem, 1)
            sync.dma_start(out=out[:], in_=o[:, :]).then_inc(dma_b, 16)
            sync.wait_ge(dma_a, 16 * 16)
            sync.wait_ge(dma_b, 17 * 16)

        @block.scalar
        def _(scalar):
            emit_gather(scalar, 1)

        @block.gpsimd
        def _(gpsimd):
            emit_gather(gpsimd, 2)

        @block.vector
        def _(vector):
            # selector for the partition-group reduce (built while DMAs fly)
            vector.memset(sel[:, :], 0.0)
            vector.memset(sel[0:64, 0:1], 1.0)
            vector.memset(sel[64:128, 1:2], 1.0)
            # rr[p, j] = sum over (t0, t4t3t2) of g at run position RPOS[j]
            gv = g[:, :].rearrange("p (t i r) -> p t i r", i=8, r=69)
            vector.wait_ge(dma_a, 16 * 16)
            for j in range(4):
                vector.tensor_reduce(
                    rr[0:64, j : j + 1],
                    gv[0:64, :, :, RPOS[j]],
                    axis=mybir.AxisListType.XY,
                    op=mybir.AluOpType.add,
                )
            vector.wait_ge(dma_b, 16 * 16)
            for j in range(4):
                ins = vector.tensor_reduce(
                    rr[64:128, j : j + 1],
                    gv[64:128, :, :, RPOS[j]],
                    axis=mybir.AxisListType.XY,
                    op=mybir.AluOpType.add,
                )
            ins.then_inc(v_sem, 1)
            vector.wait_ge(mm_sem, 1)
            vector.tensor_copy(o[:, :], acc[:, :]).then_inc(o_sem, 1)

        @block.tensor
        def _(tensor):
            # acc[j, :] = sum_{p>>6 == j} rr[p, :]
            tensor.wait_ge(v_sem, 1)
            tensor.matmul(acc[:, :], sel[:, :], rr[:, :], start=True, stop=True).then_inc(
                mm_sem, 1
            )

    nc.finalize()
    return nc


def _shard(rho):
    """core a -> rows with kept bits == a, ordered by compact traced index."""
    rm = _rowmask()
    maps = []
    for a in range(_NCORES):
        base = ((a >> 2) & 1) * _WK[0] + ((a >> 1) & 1) * _WK[1] + (a & 1) * _WK[2]
        maps.append({"x": np.ascontiguousarray(rho[base + rm, :])})
    return maps


def kernel(rho):
    global LAST_RESULTS
    from concourse.bass_utils import run_bass_kernel_spmd

    rho = np.asarray(rho, dtype=np.float32).reshape(8192, 8192)
    if "nc" not in _state:
        _state["nc"] = _build_nc()
    res = run_bass_kernel_spmd(
        _state["nc"], _shard(rho), list(range(_NCORES)), trace=TRACE
    )
    LAST_RESULTS = res
    return np.stack([res.results[a]["out"] for a in range(_NCORES)]).astype(np.float32)
